# revision 11
# baseline (speedup 1.0000x reference)
"""Chunked causal self-attention with RoPE on 8 Trainium2 NeuronCores.

Problem: B=4, L=4096, H=16, DH=DV=128, CHUNK=1024 (N=4 chunks).
RoPE is applied to q and k, then chunk-local causal attention per
(batch, chunk, head).

Sharding: heads are split across the 8 cores (2 heads/core), giving each
core B*N*heads_per_core = 32 independent (1024 x 128) attention problems.

v3 design:
  - RoPE on the host (numpy fp32) during packing; q/k/v are shipped once
    as fp16, d-major, so on-chip work is matmul + exp + small DVE sums.
  - S^T = K_i'.T @ Q' per causal k-block. The 8 blocks (causal widths
    1024,896,...,128) are packed into five 1024-column PSUM tile-groups
    {0},{1,7},{2,6},{3,5},{4} so the exp pass is 5 wide activation
    instructions per problem with zero wasted columns.
  - The diagonal block's upper triangle is prefilled to -60000 via a
    matmul-copy (eye @ msk) so exp gives exact zeros there.
  - P~ = exp(S^T/sqrt(dh)) in fp16; D = sum_i P~_i on DVE; the
    denominator (column sum of D over k) is reduced on the otherwise-idle
    GPSIMD engine (partition_all_reduce) and shipped as (1,1024) fp32.
  - out^T = sum_i V_i.T @ P~_i accumulated in a single PSUM tile, copied
    to fp16 SBUF on DVE, DMA'd out. Host divides by the denominator.
  - PE stream is software-pipelined: PV(p-1) is emitted between the S
    groups of problem p, so the tensor engine stays busy while the
    activation engine works through problem p's exps.
  - PSUM budget: 3 S-group tiles (2 banks each) + 1 PV tile (2 banks)
    = 8 banks exactly.
"""

import math
import os
import sys

import numpy as np

for _p in ("/opt/trn_rl_repo", "/root/.axon_site/_ro/trn_rl_repo"):
    if os.path.isdir(_p) and _p not in sys.path:
        sys.path.insert(0, _p)

import concourse.bass as bass  # noqa: E402
import concourse.tile as tile  # noqa: E402
from concourse import bass_isa, bass_utils, mybir  # noqa: E402

B, L, H, DH, DV = 4, 4096, 16, 128, 128
CHUNK = 1024
NCHUNK = L // CHUNK  # 4
ROPE_BASE = 10000.0
NCORES = 8
HPC = H // NCORES  # heads per core = 2
NPROB = B * HPC * NCHUNK  # 32 problems per core
HALF = DH // 2  # 64
NB = CHUNK // 128  # 8 k-blocks per chunk
SCALE = 1.0 / math.sqrt(DH)
MASK_NEG = -60000.0

F16 = mybir.dt.float16
F32 = mybir.dt.float32
AF = mybir.ActivationFunctionType

# S-block packing: (psum_tag, [(block, col_offset), ...]) per group.
# Block i covers q in [128*i, 1024): width 1024-128*i. Groups pack to
# exactly 1024 columns (except {4}: 512).
GROUPS = [
    ("sA", [(0, 0)]),
    ("sB", [(1, 0), (7, 896)]),
    ("sC", [(2, 0), (6, 768)]),
    ("sA", [(3, 0), (5, 640)]),
    ("sB", [(4, 0)]),
]


def _blk_width(i):
    return CHUNK - 128 * i


def _block_map():
    """block -> (group_index, col_offset)."""
    m = {}
    for gi, (_tag, blks) in enumerate(GROUPS):
        for (i, off) in blks:
            m[i] = (gi, off)
    return m


BLOCK_MAP = _block_map()


def _bank_pieces(a, b):
    """Split [a,b) at 512-column PSUM bank boundaries."""
    out = []
    while a < b:
        nxt = min(b, (a // 512 + 1) * 512)
        out.append((a, nxt))
        a = nxt
    return out


def build_module(nprob=NPROB):
    from concourse import bacc

    nc = bacc.Bacc("TRN2", target_bir_lowering=False, debug=False)

    qT = nc.dram_tensor("qT_in", (nprob, 128, CHUNK), F16, kind="ExternalInput")
    kT = nc.dram_tensor("kT_in", (nprob, 128, CHUNK), F16, kind="ExternalInput")
    vv = nc.dram_tensor("v_in", (nprob, 128, CHUNK), F16, kind="ExternalInput")
    msk = nc.dram_tensor("msk_in", (128, 128), F16, kind="ExternalInput")
    eye = nc.dram_tensor("eye_in", (128, 128), F16, kind="ExternalInput")

    outT = nc.dram_tensor("outT_out", (nprob, 128, CHUNK), F16, kind="ExternalOutput")
    den = nc.dram_tensor("den_out", (nprob, CHUNK), F32, kind="ExternalOutput")

    with tile.TileContext(nc) as tc:
        _body(tc, nprob, qT, kT, vv, msk, eye, outT, den)
    nc.compile()
    return nc


def _body(tc, nprob, qT, kT, vv, msk, eye, outT, den):
    from contextlib import ExitStack

    nc = tc.nc
    with ExitStack() as ctx:
        singles = ctx.enter_context(tc.tile_pool(name="singles", bufs=1))
        io = ctx.enter_context(tc.tile_pool(name="io", bufs=4))
        ptp = ctx.enter_context(tc.tile_pool(name="ptp", bufs=2))
        dp = ctx.enter_context(tc.tile_pool(name="dp", bufs=2))
        psS = ctx.enter_context(tc.tile_pool(name="psS", bufs=1, space="PSUM"))
        psOp = ctx.enter_context(tc.tile_pool(name="psO", bufs=1, space="PSUM"))

        msk_t = singles.tile([128, 128], F16, tag="msk")
        nc.sync.dma_start(out=msk_t, in_=msk.ap())
        eye_t = singles.tile([128, 128], F16, tag="eye")
        nc.sync.dma_start(out=eye_t, in_=eye.ap())
        # Touch consts once so later PE ops don't carry the DMA waits.
        dummy = singles.tile([128, 1], F16, tag="dummy")
        nc.vector.tensor_copy(out=dummy, in_=msk_t[:, 0:1])
        nc.vector.tensor_copy(out=dummy, in_=eye_t[:, 0:1])

        def emit_group(tq, tk, gi):
            """S^T matmuls + exp for group gi; returns the P~ tile."""
            tag, blks = GROUPS[gi]
            width = sum(_blk_width(i) for (i, _off) in blks)
            ps = psS.tile([128, CHUNK], F32, tag=tag)
            for (i, off) in blks:
                kblk = tk[:, 128 * i:128 * i + 128]
                # Diagonal block at tile cols [off, off+128).
                nc.tensor.matmul(
                    ps[:, off:off + 128], lhsT=eye_t, rhs=msk_t,
                    start=True, stop=False,
                )
                nc.tensor.matmul(
                    ps[:, off:off + 128], lhsT=kblk,
                    rhs=tq[:, 128 * i:128 * i + 128],
                    start=False, stop=True,
                )
                for (a, b) in _bank_pieces(off + 128, off + _blk_width(i)):
                    q0 = a - off + 128 * i
                    nc.tensor.matmul(
                        ps[:, a:b], lhsT=kblk, rhs=tq[:, q0:q0 + (b - a)],
                        start=True, stop=True,
                    )
            pt = ptp.tile([128, CHUNK], F16, tag="pt" + str(gi))
            nc.scalar.activation(
                out=pt[:, 0:width], in_=ps[:, 0:width], func=AF.Exp,
                scale=SCALE,
            )
            return pt

        def emit_pv(prev):
            pts, tv, d_tile, p = prev
            pso = psOp.tile([128, CHUNK], F32, tag="psO")
            for i in range(NB):
                gi, off = BLOCK_MAP[i]
                vblk = tv[:, 128 * i:128 * i + 128]
                for (a, b) in _bank_pieces(128 * i, CHUNK):
                    stop = (i == 3 and b == 512) or (i == NB - 1)
                    c0 = a - 128 * i + off
                    nc.tensor.matmul(
                        pso[:, a:b], lhsT=vblk, rhs=pts[gi][:, c0:c0 + (b - a)],
                        start=(i == 0), stop=stop,
                    )
            outf = dp.tile([128, CHUNK], F16, tag="outf")
            nc.vector.tensor_copy(out=outf, in_=pso)
            nc.sync.dma_start(out=outT.ap()[p], in_=outf)

        prev = None

        for p in range(nprob):
            tq = io.tile([128, CHUNK], F16, tag="q")
            nc.sync.dma_start(out=tq, in_=qT.ap()[p])
            tk = io.tile([128, CHUNK], F16, tag="k")
            nc.sync.dma_start(out=tk, in_=kT.ap()[p])
            tv = io.tile([128, CHUNK], F16, tag="v")
            nc.sync.dma_start(out=tv, in_=vv.ap()[p])

            pts = []
            for gi in range(4):
                pts.append(emit_group(tq, tk, gi))
            # PV of the previous problem late in this problem's S groups
            # keeps PE busy while Act drains this problem's exps, and
            # lets exp(A2) start as soon as its matmuls finish.
            if prev is not None:
                emit_pv(prev)
            pts.append(emit_group(tq, tk, 4))

            # D = sum_i P~_i over absolute q columns (DVE).
            d_tile = dp.tile([128, CHUNK], F16, tag="D")
            nc.vector.tensor_copy(out=d_tile, in_=pts[0])
            for i in range(1, NB):
                gi, off = BLOCK_MAP[i]
                q0 = 128 * i
                w = _blk_width(i)
                nc.vector.tensor_add(
                    d_tile[:, q0:CHUNK], d_tile[:, q0:CHUNK],
                    pts[gi][:, off:off + w],
                )
            # Column sum over k partitions on GPSIMD; row 0 holds den.
            denf = dp.tile([128, CHUNK], F32, tag="denf")
            nc.gpsimd.partition_all_reduce(
                denf, d_tile, channels=128, reduce_op=bass_isa.ReduceOp.add
            )
            nc.sync.dma_start(out=den.ap()[p], in_=denf[0:1, :])

            prev = (pts, tv, d_tile, p)

        emit_pv(prev)


def _host_rope(q, k):
    half = HALF
    freqs = np.exp(np.arange(half, dtype=np.float64) * (-math.log(ROPE_BASE) / half))
    ang = np.arange(L, dtype=np.float64)[:, None] * freqs[None, :]  # (L, 64)
    cos = np.cos(ang).astype(np.float32)[:, None, :]  # (L,1,64)
    sin = np.sin(ang).astype(np.float32)[:, None, :]

    def rope(x):
        x1, x2 = x[..., :half], x[..., half:]
        return np.concatenate([x1 * cos - x2 * sin, x2 * cos + x1 * sin], axis=-1)

    return rope(q), rope(k)


def _pack_core(qc, kc, vc):
    """qc,kc,vc: (B, L, HPC, 128) fp32 slices for one core -> input dict."""

    def dmaj(x):
        return np.ascontiguousarray(
            x.transpose(0, 2, 1, 3)
            .reshape(B, HPC, NCHUNK, CHUNK, DH)
            .transpose(0, 1, 2, 4, 3)
        ).astype(np.float16).reshape(NPROB, DH, CHUNK)

    qT = dmaj(qc)
    kT = dmaj(kc)
    # v: (B, L, h, D) -> (B, h, N, NB, 128, D) -> (B, h, N, 128, NB, D)
    vp = (
        vc.transpose(0, 2, 1, 3)
        .reshape(B, HPC, NCHUNK, NB, 128, DV)
        .transpose(0, 1, 2, 4, 3, 5)
    )
    vp = np.ascontiguousarray(vp).astype(np.float16).reshape(NPROB, 128, CHUNK)
    return dict(qT_in=qT, kT_in=kT, v_in=vp)


def _host_consts():
    r = np.arange(128)
    msk = np.where(r[:, None] <= r[None, :], 0.0, MASK_NEG).astype(np.float16)
    eye = np.eye(128, dtype=np.float16)
    return msk, eye


_NC_CACHE = {}
LAST_RESULT = None


def _get_module(nprob=NPROB):
    if nprob not in _NC_CACHE:
        _NC_CACHE[nprob] = build_module(nprob)
    return _NC_CACHE[nprob]


def prepare(q, k, v):
    """Build (nc, in_maps) for the 8 cores from full fp32 inputs."""
    q = np.asarray(q, dtype=np.float32)
    k = np.asarray(k, dtype=np.float32)
    v = np.asarray(v, dtype=np.float32)

    qr, kr = _host_rope(q, k)
    msk, eye = _host_consts()
    consts = dict(msk_in=msk, eye_in=eye)

    in_maps = []
    for c in range(NCORES):
        hs = slice(HPC * c, HPC * (c + 1))
        m = _pack_core(qr[:, :, hs], kr[:, :, hs], v[:, :, hs])
        m.update(consts)
        in_maps.append(m)
    return _get_module(NPROB), in_maps


def unpack_output(results):
    """results: list of per-core dicts with outT_out/den_out -> (B,L,H,DV)."""
    out = np.empty((B, L, H, DV), np.float32)
    for c in range(NCORES):
        ot = results[c]["outT_out"].astype(np.float32)  # (32, 128dv, 1024q)
        den = results[c]["den_out"].astype(np.float32)  # (32, 1024q)
        o = ot / den[:, None, :]
        # (B, h, N, dv, q) -> (B, N, q, h, dv)
        o = o.reshape(B, HPC, NCHUNK, DV, CHUNK).transpose(0, 2, 4, 1, 3)
        out[:, :, HPC * c:HPC * (c + 1)] = o.reshape(B, L, HPC, DV)
    return out


def kernel(q, k, v):
    nc, in_maps = prepare(q, k, v)
    trace = bool(int(os.environ.get("KERNEL_TRACE", "0")))
    res = bass_utils.run_bass_kernel_spmd(
        nc, in_maps, core_ids=list(range(NCORES)), trace=trace
    )
    global LAST_RESULT
    LAST_RESULT = res
    return unpack_output(res.results)
